# revision 43
# baseline (speedup 1.0000x reference)
"""CDD loss kernel for 8 Trainium2 NeuronCores (Bass/Tile, SPMD).

Math (validated vs reference in float32):
  ps is one-hot -> every (C,C,N,N) reference tensor collapses to per-class-
  block sums. Host sorts+pads src rows by class (CAP rows/class, pads are
  huge distinct sentinel vectors so exp(-dist/bw) underflows to exactly 0).
  The E_pp class-diagonal blocks have their diagonal zeroed on device, making
  each diagonal entry contribute exactly exp(0)=1 per bandwidth; the exact
  correction (5*CAP - 5*exp(-1e-5)*cs) is applied as a host-computed offset.
  g2 is symmetric -> T2 = T1^T, so inter = sum_{s!=t} 2*(T1-T3)/(C^2-C).

Distribution (SPMD, one program, per-core data):
  - every core computes E rows for its class pair (rotation of the padded
    src rows makes "own" rows/cols sit at fixed offsets), partial
    S1 = Wown^T E_pp W, sst = Wown^T E_pt pt, stt = pt^T E_tt pt / 8
  - one AllReduce of the packed [12,36] partials
  - gammas + negative inverse bandwidths on device (tiny DVE ops)
  - exp-heavy sums: T1/k1 and T3 run as single ACT instructions over
    flattened broadcast tiles with per-partition scale and accum_out
    (free-dim reduction inside the ACT op); k2/k3 as [128,*] passes
  - per-core weighted reduce with host weight matrix -> [intra, inter]
    partials, host sums the 8 partials.
"""

import math
import numpy as np

C = 12
KN = 5
MU = 2
N = 384
D = 256
CAP = 64
R = C * CAP            # 768 padded src rows
NCORES = 8
NCOL = 22              # ACC columns: T1, T3, k1*5, k3*5, k2*(2q x 5k)
DIAG5 = 5.0 * math.exp(-1e-5)
I2 = 2.0 / (C * C - C)

_COMPILED = {}


# ----------------------------------------------------------------------------
# host-side prep
# ----------------------------------------------------------------------------

def _host_prep(src_x, tgt_x, src_y, tgt_y):
    src_x = np.ascontiguousarray(np.asarray(src_x, dtype=np.float32))
    tgt_x = np.ascontiguousarray(np.asarray(tgt_x, dtype=np.float32))
    src_y = np.asarray(src_y).astype(np.int64)
    pt = np.ascontiguousarray(np.asarray(tgt_y, dtype=np.float32))

    counts = np.bincount(src_y, minlength=C)
    if counts.max() > CAP:
        return None  # caller falls back to numpy path

    perm = np.argsort(src_y, kind="stable")
    sx_pad = np.zeros((R, D), np.float32)
    W = np.zeros((R, C), np.float32)
    # pad sentinels: huge random-sign vectors. Pad-pad dot products are then
    # tiny relative to the norms (no catastrophic cancellation in d2), every
    # pad-involved distance is >= ~3e5 and exp(-dist/bw) underflows to 0.
    rng = np.random.default_rng(987654321)
    sgn = (rng.integers(0, 2, size=(R, D)).astype(np.float32) * 2.0 - 1.0)
    off = 0
    padidx = 0
    for c in range(C):
        idx = perm[off:off + counts[c]]
        sx_pad[c * CAP:c * CAP + counts[c]] = src_x[idx]
        W[c * CAP:c * CAP + counts[c], c] = 1.0
        for p in range(CAP - counts[c]):
            sx_pad[c * CAP + counts[c] + p, :] = 2.0e4 * sgn[padidx]
            padidx += 1
        off += counts[c]

    cs = counts.astype(np.float64)
    ct = pt.sum(0).astype(np.float64)
    pss = cs * cs
    ptt = ct * ct

    rden2 = (1.0 / (pss[:, None] + pss[None, :]
                    + 2.0 * cs[:, None] * cs[None, :])).astype(np.float32)
    rdenin = (1.0 / (pss + ptt + 2.0 * cs * ct)).astype(np.float32).reshape(C, 1)

    eye128 = np.eye(128, dtype=np.float32)
    diagm = np.concatenate([1.0 - np.eye(CAP, dtype=np.float32)] * 2, axis=0)
    eye12 = np.eye(C, dtype=np.float32)
    pw60 = np.zeros((C, 60), np.float32)
    for k in range(KN):
        pw60[:, k * 12:(k + 1) * 12] = -(float(MU) ** (k - KN // 2))
    pw5 = np.zeros((C, 5), np.float32)
    for k in range(KN):
        pw5[:, k] = -(float(MU) ** (k - KN // 2))
    ones128 = np.ones((128, 1), np.float32)
    ssel = np.zeros((NCOL, 2), np.float32)
    ssel[2:NCOL, 0] = 1.0  # intra cols: k1 (2-6), k3 (7-11), k2 (12-21)
    ssel[0:2, 1] = 1.0    # inter cols: T1, T3

    in_maps = []
    for r in range(NCORES):
        g = r % 6
        a, b = 2 * g, 2 * g + 1
        pp_active = r < 6
        roll = 2 * g * CAP

        sxf = np.ascontiguousarray(np.roll(sx_pad, -roll, axis=0))
        wr = np.ascontiguousarray(np.roll(W, -roll, axis=0))
        wown = wr[0:128].copy() if pp_active else np.zeros((128, C), np.float32)

        oh2 = np.zeros((C, 2), np.float32)
        oh2[a, 0] = 1.0
        oh2[b, 1] = 1.0

        k2cls = []
        for q in range(2):
            c = r + 8 * q
            k2cls.append(c if c < C else -1)
        k2abs = [c if c >= 0 else 0 for c in k2cls]
        k2sel = np.zeros((C, 2), np.float32)
        ptrow2 = np.zeros((2, N), np.float32)
        ptcolf = np.zeros((128, 6), np.float32)
        for q, c in enumerate(k2cls):
            cc = c if c >= 0 else 0
            k2sel[cc, q] = 1.0
            ptrow2[q] = pt[:, cc]
            for blk in range(3):
                ptcolf[:, q * 3 + blk] = pt[blk * 128:(blk + 1) * 128, cc]

        ptr3a = pt[:, a].reshape(1, N).astype(np.float32)
        ptr3b = pt[:, b].reshape(1, N).astype(np.float32)

        # reindex matrix for the T3 scale column:
        # dest t*5+k <- source k*12 + rot(t) with rot(t) = (2g+t) % 12
        # reorder for sclT3: dest row (within half) = k*12 + t (k-major,
        # t = rotated position) <- source k*12 + absolute class
        perm65 = np.zeros((65, 65), np.float32)
        for t in range(12):
            for k in range(KN):
                perm65[k * 12 + ((2 * g + t) % 12), k * 12 + t] = 1.0
        for j in range(60, 65):
            perm65[j, j] = 1.0

        wm = np.zeros((128, NCOL), np.float32)
        if pp_active:
            for h, cls in ((0, a), (1, b)):
                for k in range(KN):
                    for t in range(12):
                        if t != cls:
                            wm[h * 64 + k * 12 + t, 0] = I2 / pss[cls]
                for t in range(12):
                    rt_ = (2 * g + t) % 12
                    if rt_ != cls:
                        for k in range(KN):
                            wm[h * 64 + k * 12 + t, 1] = -I2 / (cs[cls] * cs[rt_])
                for k in range(KN):
                    wm[h * CAP:(h + 1) * CAP, 2 + k] = 1.0 / (C * pss[cls])
                    wm[h * CAP:(h + 1) * CAP, 7 + k] = -2.0 / (C * cs[cls] * ct[cls])
        for q, c in enumerate(k2cls):
            if c >= 0:
                for k in range(KN):
                    wm[:, 12 + q * KN + k] = 1.0 / (C * ptt[c])

        offs = np.zeros((1, 2), np.float32)
        if r == 0:
            corr = 5.0 * CAP - DIAG5 * cs
            offs[0, 0] = -(corr / pss / C).sum()
            offs[0, 1] = -((C - 1) * corr * I2 / pss).sum()

        # ---- packed input tensors (few big DMAs instead of ~36 small) ----
        sxp = np.ascontiguousarray(
            sxf.reshape(6, 128, D).transpose(1, 0, 2).reshape(128, 6 * D))
        txp = np.ascontiguousarray(
            tgt_x.reshape(3, 128, D).transpose(1, 0, 2).reshape(128, 3 * D))
        pwp = np.zeros((128, 377), np.float32)
        pwp[:, 0:36] = pt.reshape(3, 128, C).transpose(1, 0, 2).reshape(128, 36)
        pwp[:, 36:108] = wr.reshape(6, 128, C).transpose(1, 0, 2).reshape(128, 72)
        pwp[:, 108:120] = wown
        ptr4 = np.concatenate(
            [ptrow2[0], ptrow2[1], ptr3a[0], ptr3b[0]]).reshape(1, 4 * N)
        # k2-class src rows (unrotated), pre-transposed, for the local sst
        # diag recompute that replaces the collective
        xk2 = np.concatenate([sx_pad[c0k * CAP:(c0k + 1) * CAP]
                              for c0k in k2abs], axis=0)  # [128, D]
        pwp[:, 120:248] = xk2[:, 0:128].T
        pwp[:, 248:376] = xk2[:, 128:256].T
        pwp[:, 376] = (xk2 * xk2).sum(1)
        # locals replacing the collective: ptk2w (per-half pt cols for the
        # k2 sst rows), wk2f (real-row selectors), mab (own-class diag mask),
        # sck2 (scatter of sstk2 into the [12] diag, deduped vs own classes)
        ptk2w = np.zeros((128, N), np.float32)
        ptk2w[0:64] = pt[:, k2abs[0]]
        ptk2w[64:128] = pt[:, k2abs[1]]
        wk2f = np.zeros((128, 2), np.float32)
        mab = np.zeros((C, 1), np.float32)
        sck2 = np.zeros((2, C), np.float32)
        own = {a, b} if pp_active else set()
        mab[list(own) if own else [], 0] = 1.0
        for q, c in enumerate(k2cls):
            cq = k2abs[q]
            wk2f[q * 64:q * 64 + counts[cq], q] = 1.0
            if c >= 0 and c not in own:
                sck2[q, c] = 1.0
        cpk = np.zeros((128, 797), np.float32)
        cpk[:, 376:760] = ptk2w
        cpk[:, 760:762] = wk2f
        cpk[0:C, 762:763] = mab
        cpk[0:2, 763:775] = sck2
        cpk[:, 0:128] = eye128
        cpk[:, 128:192] = diagm
        cpk[:, 192:198] = ptcolf
        cpk[:, 775:797] = wm
        cpk[:, 212:213] = ones128
        cpk[0:12, 213:225] = eye12
        cpk[0:12, 225:227] = oh2
        cpk[0:12, 227:229] = k2sel
        cpk[0:12, 229:289] = pw60
        cpk[0:12, 289:294] = pw5
        cpk[0:12, 294:306] = rden2
        cpk[0:12, 306:307] = rdenin
        cpk[0:65, 307:372] = perm65
        cpk[0:NCOL, 372:374] = ssel
        cpk[0:1, 374:376] = offs
        in_maps.append({"sxp": sxp, "txp": txp, "pwp": pwp, "cpk": cpk,
                        "ptr4": ptr4})
    return in_maps


def _numpy_fallback(src_x, tgt_x, src_y, tgt_y):
    f = np.float32
    src_x = np.asarray(src_x, f)
    tgt_x = np.asarray(tgt_x, f)
    src_y = np.asarray(src_y).astype(np.int64)
    pt = np.asarray(tgt_y, f)
    ps = np.eye(C, dtype=f)[src_y]

    def cdist(a, bb):
        d2 = (a * a).sum(1)[:, None] + (bb * bb).sum(1)[None, :] - 2.0 * (a @ bb.T)
        return np.sqrt(np.maximum(d2, 0.0))

    def kern(dist, g):
        acc = 0.0
        for i in range(KN):
            bw = np.maximum(np.asarray(g) * (MU ** (i - KN // 2)), 1e-5)
            acc = acc + np.exp(-np.clip(dist / bw, 1e-5, 1e5))
        return acc

    E_ss = cdist(src_x, src_x); E_tt = cdist(tgt_x, tgt_x); E_st = cdist(src_x, tgt_x)
    sss = np.einsum('ic,ij,jc->c', ps, E_ss, ps)
    stt = np.einsum('ic,ij,jc->c', pt, E_tt, pt)
    sst = np.einsum('is,ij,jt->st', ps, E_st, pt)
    cs = ps.sum(0); ct = pt.sum(0)
    pss = cs * cs; ptt = ct * ct; pstd = cs * ct
    g_in = (sss + stt + 2 * np.diagonal(sst)) / (pss + ptt + 2 * pstd)
    Pss = ps.T[:, :, None] * ps.T[:, None, :]
    Ptt = pt.T[:, :, None] * pt.T[:, None, :]
    Pst = ps.T[:, :, None] * pt.T[:, None, :]
    k1 = (kern(E_ss[None] * Pss, g_in[:, None, None]) * Pss).sum((-2, -1)) / pss
    k2 = (kern(E_tt[None] * Ptt, g_in[:, None, None]) * Ptt).sum((-2, -1)) / ptt
    k3 = (kern(E_st[None] * Pst, g_in[:, None, None]) * Pst).sum((-2, -1)) / pstd
    intra = (k1 + k2 - 2 * k3).sum() / C
    sst_s = np.einsum('is,ij,jt->st', ps, E_ss, ps)
    g2 = (sss[:, None] + sss[None, :] + 2 * sst_s) / (
        pss[:, None] + pss[None, :] + 2 * cs[:, None] * cs[None, :])
    T1 = np.zeros((C, C), f); T3 = np.zeros((C, C), f)
    for s in range(C):
        ms = ps[:, s].astype(bool)
        for t in range(C):
            mt = ps[:, t].astype(bool)
            T1[s, t] = kern(E_ss[np.ix_(ms, ms)], g2[s, t]).sum() / pss[s]
            T3[s, t] = kern(E_ss[np.ix_(ms, mt)], g2[s, t]).sum() / (cs[s] * cs[t])
    inter = ((2 * T1 - 2 * T3) * (1 - np.eye(C))).sum() / (C * C - C)
    return np.array([intra, inter], np.float32)


# ----------------------------------------------------------------------------
# device program
# ----------------------------------------------------------------------------

def _build_program():
    import os
    import concourse.bass as bass
    import concourse.tile as tile
    from concourse import bacc, mybir

    STAGE = int(os.environ.get("CDD_STAGE", "99"))

    f32 = mybir.dt.float32
    AF = mybir.ActivationFunctionType
    OP = mybir.AluOpType

    nc = bacc.Bacc("TRN2", target_bir_lowering=False, debug=False,
                   num_devices=NCORES)

    def din(name, shape):
        return nc.dram_tensor(name, list(shape), f32, kind="ExternalInput").ap()

    i_sxp = din("sxp", (128, 6 * D))
    i_txp = din("txp", (128, 3 * D))
    i_pwp = din("pwp", (128, 377))
    i_cpk = din("cpk", (128, 797))
    i_ptr4 = din("ptr4", (1, 4 * N))

    o_out = nc.dram_tensor("out", [1, 2], f32, kind="ExternalOutput").ap()

    with tile.TileContext(nc) as tc:
        with (
            tc.tile_pool(name="io", bufs=1) as io,
            tc.tile_pool(name="big", bufs=1) as big,
            tc.tile_pool(name="scr", bufs=2) as scr,
            tc.tile_pool(name="sm", bufs=1) as sm,
            tc.tile_pool(name="pG", bufs=2, space="PSUM") as pG,
            tc.tile_pool(name="pA", bufs=1, space="PSUM") as pA,
            tc.tile_pool(name="pT", bufs=2, space="PSUM") as pT,
            tc.tile_pool(name="pS", bufs=1, space="PSUM") as pS,
            tc.tile_pool(name="pS2", bufs=1, space="PSUM") as pS2,
            tc.tile_pool(name="dram", bufs=1, space="DRAM") as dpool,
        ):
            dma = nc.sync.dma_start
            dma2 = nc.scalar.dma_start

            def load(name, ap_in, shape):
                t = io.tile(list(shape), f32, tag=name, name=name)
                dma(out=t[:], in_=ap_in[:])
                return t

            t_sxp = load("sxp", i_sxp, (128, 6 * D))
            t_txp = load("txp", i_txp, (128, 3 * D))
            t_pwp = load("pwp", i_pwp, (128, 377))
            t_cpk = load("cpk", i_cpk, (128, 797))
            t_ptr4 = load("ptr4", i_ptr4, (1, 4 * N))

            sxf = [t_sxp[:, m * D:(m + 1) * D] for m in range(6)]
            tx = [t_txp[:, m * D:(m + 1) * D] for m in range(3)]
            ptb = [t_pwp[:, b * C:(b + 1) * C] for b in range(3)]
            wrb = [t_pwp[:, 36 + m * C:36 + (m + 1) * C] for m in range(6)]
            wown = t_pwp[:, 108:120]
            eye128 = t_cpk[:, 0:128]
            diagm = t_cpk[:, 128:192]
            ptcolf = t_cpk[:, 192:198]
            wm = t_cpk[:, 775:797]
            ones = t_cpk[:, 212:213]
            eye12 = t_cpk[0:C, 213:225]
            oh2 = t_cpk[0:C, 225:227]
            k2sel = t_cpk[0:C, 227:229]
            pw60 = t_cpk[0:C, 229:289]
            pw5 = t_cpk[0:C, 289:294]
            rden2 = t_cpk[0:C, 294:306]
            rdenin = t_cpk[0:C, 306:307]
            perm65 = t_cpk[0:65, 307:372]
            ssel = t_cpk[0:NCOL, 372:374]
            offs = t_cpk[0:1, 374:376]
            ptr2 = [t_ptr4[0:1, 0:N], t_ptr4[0:1, N:2 * N]]
            ptr3 = [t_ptr4[0:1, 2 * N:3 * N], t_ptr4[0:1, 3 * N:4 * N]]
            k2xT = [t_pwp[:, 120:248], t_pwp[:, 248:376]]
            rk2col = t_pwp[:, 376:377]
            ptk2w = t_cpk[:, 376:760]
            wk2f = t_cpk[:, 760:762]
            mab = t_cpk[0:C, 762:763]
            sck2m = t_cpk[0:2, 763:775]

            if STAGE >= 11:
                # ---------------- transposes: sxfT, txT ----------------
                sxfT = [big.tile([128, R], f32, tag=f"sxfT{k}", name=f"sxfT{k}")
                        for k in range(2)]
                txT = [big.tile([128, N], f32, tag=f"txT{k}", name=f"txT{k}")
                       for k in range(2)]
                for m in range(6):
                    for k in range(2):
                        tp_ = pT.tile([128, 128], f32, tag="tiny", name="tp")
                        nc.tensor.transpose(tp_[:], sxf[m][:, k * 128:(k + 1) * 128],
                                            eye128[:])
                        nc.vector.tensor_copy(sxfT[k][:, m * 128:(m + 1) * 128], tp_[:])
                for m in range(3):
                    for k in range(2):
                        tp_ = pT.tile([128, 128], f32, tag="tiny", name="tp")
                        nc.tensor.transpose(tp_[:], tx[m][:, k * 128:(k + 1) * 128],
                                            eye128[:])
                        nc.vector.tensor_copy(txT[k][:, m * 128:(m + 1) * 128], tp_[:])

            if STAGE >= 12:
                # ---------------- row norms ----------------
                rscol = [sm.tile([128, 1], f32, tag=f"rs{m}", name=f"rs{m}")
                         for m in range(6)]
                rtcol = [sm.tile([128, 1], f32, tag=f"rt{m}", name=f"rt{m}")
                         for m in range(3)]
                for m in range(6):
                    nsc = scr.tile([128, D], f32, tag="normscr", name="nsc")
                    nc.scalar.activation(nsc[:], sxf[m][:], AF.Square,
                                         accum_out=rscol[m][:])
                for m in range(3):
                    nsc = scr.tile([128, D], f32, tag="normscr", name="nsc")
                    nc.scalar.activation(nsc[:], tx[m][:], AF.Square,
                                         accum_out=rtcol[m][:])

                rsrow = sm.tile([1, R], f32, tag="rsrow", name="rsrow")
                rtrow = sm.tile([1, N], f32, tag="rtrow", name="rtrow")
                for m in range(6):
                    tp_ = pT.tile([1, 128], f32, tag="tiny", name="tpr")
                    nc.tensor.transpose(tp_[:], rscol[m][:], eye128[:])
                    nc.vector.tensor_copy(rsrow[:, m * 128:(m + 1) * 128], tp_[:])
                for m in range(3):
                    tp_ = pT.tile([1, 128], f32, tag="tiny", name="tpr")
                    nc.tensor.transpose(tp_[:], rtcol[m][:], eye128[:])
                    nc.vector.tensor_copy(rtrow[:, m * 128:(m + 1) * 128], tp_[:])

                rsrowb = big.tile([128, R], f32, tag="rsrowb", name="rsrowb")
                rtrowb = big.tile([128, N], f32, tag="rtrowb", name="rtrowb")
                nc.gpsimd.partition_broadcast(rsrowb[:], rsrow[:])
                nc.gpsimd.partition_broadcast(rtrowb[:], rtrow[:])

            if STAGE >= 13:
                # ---------------- E matrices ----------------
                def emit_E(dst, lhsT_tiles, lhs_lo, rhs_tiles, n_cols, rcol, rowb):
                    done = 0
                    while done < n_cols:
                        nchunk = min(512, n_cols - done)
                        gp = pG.tile([128, 512], f32, tag="G", name="gp")
                        for k in range(2):
                            nc.tensor.matmul(
                                gp[:, :nchunk],
                                lhsT_tiles[k][:, lhs_lo:lhs_lo + 128],
                                rhs_tiles[k][:, done:done + nchunk],
                                start=(k == 0), stop=(k == 1))
                        t1_ = scr.tile([128, 512], f32, tag="d2scr", name="d2s")
                        nc.vector.scalar_tensor_tensor(
                            out=t1_[:, :nchunk], in0=gp[:, :nchunk], scalar=-2.0,
                            in1=rowb[:, done:done + nchunk],
                            op0=OP.mult, op1=OP.add)
                        nc.vector.tensor_scalar(
                            t1_[:, :nchunk], t1_[:, :nchunk],
                            rcol[:], 0.0, OP.add, OP.max)
                        nc.scalar.activation(dst[:, done:done + nchunk],
                                             t1_[:, :nchunk], AF.Sqrt)
                        done += nchunk

                E_own = big.tile([128, R], f32, tag="E_own", name="E_own")
                emit_E(E_own, sxfT, 0, sxfT, R, rscol[0], rsrowb)

                E_ttf = big.tile([128, 3 * N], f32, tag="E_ttf", name="E_ttf")
                for blk in range(3):
                    emit_E(E_ttf[:, blk * N:(blk + 1) * N], txT, blk * 128, txT, N,
                           rtcol[blk], rtrowb)

                E_pt = big.tile([128, N], f32, tag="E_pt", name="E_pt")
                emit_E(E_pt, sxfT, 0, txT, N, rscol[0], rtrowb)

                # all 6 class-pair diagonal blocks of E_ss (for local sss)
                E_dall = big.tile([128, R], f32, tag="E_dall", name="E_dall")
                for m in range(6):
                    gp = pG.tile([128, 512], f32, tag="G", name="gpd")
                    for k in range(2):
                        nc.tensor.matmul(
                            gp[:, 0:128],
                            sxfT[k][:, m * 128:(m + 1) * 128],
                            sxfT[k][:, m * 128:(m + 1) * 128],
                            start=(k == 0), stop=(k == 1))
                    td = scr.tile([128, 512], f32, tag="d2scr", name="d2sd")
                    nc.vector.scalar_tensor_tensor(
                        out=td[:, 0:128], in0=gp[:, 0:128], scalar=-2.0,
                        in1=rsrowb[:, m * 128:(m + 1) * 128],
                        op0=OP.mult, op1=OP.add)
                    nc.vector.tensor_scalar(
                        td[:, 0:128], td[:, 0:128], rscol[m][:], 0.0,
                        OP.add, OP.max)
                    nc.scalar.activation(E_dall[:, m * 128:(m + 1) * 128],
                                         td[:, 0:128], AF.Sqrt)

                # E rows for this core's k2 classes vs tgt (for local sst)
                E_k2 = big.tile([128, N], f32, tag="E_k2", name="E_k2")
                emit_E(E_k2, k2xT, 0, txT, N, rk2col, rtrowb)

            if STAGE >= 30:
                # ---------------- partial sums + collective ----------------
                part = sm.tile([C, 36], f32, tag="part", name="part")

                def small_chain(lhs_tile, rhs_ap, n_free, rhs2_tiles, acc_ps,
                                first, last):
                    ap_ = pA.tile([C, 768], f32, tag="A", name="ap_")
                    done = 0
                    while done < n_free:
                        nchunk = min(512, n_free - done)
                        nc.tensor.matmul(ap_[:, done:done + nchunk], lhs_tile[:],
                                         rhs_ap[:, done:done + nchunk],
                                         start=True, stop=True)
                        done += nchunk
                    asb = scr.tile([C, 768], f32, tag="Asb", name="asb")
                    nc.scalar.copy(asb[:, :n_free], ap_[:, :n_free])
                    nblk = n_free // 128
                    for m in range(nblk):
                        tp_ = pT.tile([128, C], f32, tag="tiny", name="tpA")
                        nc.tensor.transpose(tp_[:], asb[:, m * 128:(m + 1) * 128],
                                            eye12[:])
                        atsb = scr.tile([128, C], f32, tag="ATsb", name="atsb")
                        nc.vector.tensor_copy(atsb[:], tp_[:])
                        nc.tensor.matmul(acc_ps[:], atsb[:], rhs2_tiles[m][:],
                                         start=(first and m == 0),
                                         stop=(last and m == nblk - 1))

                s1ps = pS.tile([C, C], f32, tag="S", name="s1ps")
                small_chain(wown, E_own, R, wrb, s1ps, True, True)
                nc.vector.tensor_copy(part[:, 0:12], s1ps[:])

                stps = pS.tile([C, C], f32, tag="S", name="stps")
                for blk in range(3):
                    small_chain(ptb[blk], E_ttf[:, blk * N:(blk + 1) * N], N, ptb,
                                stps, blk == 0, blk == 2)
                nc.vector.tensor_copy(part[:, 12:24], stps[:])

                ssps = pS.tile([C, C], f32, tag="S", name="ssps")
                small_chain(wown, E_pt, N, ptb, ssps, True, True)
                nc.vector.tensor_copy(part[:, 24:36], ssps[:])

                # local sss for ALL classes: sum_m Wm^T E_dall_m Wm is block-
                # diagonal; its diagonal is sss. (Replaces the AllReduce.)
                ssall_ps = pS2.tile([C, C], f32, tag="SS", name="ssall_ps")
                for m in range(6):
                    ewp = pT.tile([128, C], f32, tag="tiny", name="ewp")
                    nc.tensor.matmul(ewp[:], E_dall[:, m * 128:(m + 1) * 128],
                                     wrb[m], start=True, stop=True)
                    ewsb = scr.tile([128, C], f32, tag="ewsb", name="ewsb")
                    nc.vector.tensor_copy(ewsb[:], ewp[:])
                    nc.tensor.matmul(ssall_ps[:], ewsb[:], wrb[m],
                                     start=(m == 0), stop=(m == 5))
                ssall = sm.tile([C, C], f32, tag="ssall", name="ssall")
                nc.vector.tensor_copy(ssall[:], ssall_ps[:])

                # local sst diag for the k2 classes
                zjunk = scr.tile([128, N], f32, tag="zjunk", name="zjunk")
                zred = sm.tile([128, 1], f32, tag="zred", name="zred")
                nc.vector.scalar_tensor_tensor(
                    out=zjunk[:], in0=E_k2[:], scalar=1.0, in1=ptk2w,
                    op0=OP.mult, op1=OP.mult, accum_out=zred[:])
                sstk2ps = pT.tile([2, 1], f32, tag="tiny", name="sstk2ps")
                nc.tensor.matmul(sstk2ps[:], wk2f, zred[:],
                                 start=True, stop=True)
                sstk2sb = sm.tile([2, 1], f32, tag="sstk2sb", name="sstk2sb")
                nc.vector.tensor_copy(sstk2sb[:], sstk2ps[:])
                sckps = pT.tile([C, 1], f32, tag="tiny", name="sckps")
                nc.tensor.matmul(sckps[:], sck2m, sstk2sb[:],
                                 start=True, stop=True)
                scksb = sm.tile([C, 1], f32, tag="scksb", name="scksb")
                nc.vector.tensor_copy(scksb[:], sckps[:])

                sred = part

            if STAGE >= 20:
                # diag-zeroed own-class diagonal blocks [128, 64]
                E_diag = big.tile([128, CAP], f32, tag="E_diag", name="E_diag")
                nc.vector.tensor_tensor(E_diag[0:CAP, :], E_own[0:CAP, 0:CAP],
                                        diagm[0:CAP, :], OP.mult)
                nc.vector.tensor_tensor(E_diag[CAP:128, :],
                                        E_own[CAP:128, CAP:128],
                                        diagm[CAP:128, :], OP.mult)

                # E -> DRAM for the flat broadcast reads (ACT HWDGE ring, so
                # these never queue ahead of the collective input on SP ring).
                # d_eo2 stores each 64x64 class block contiguously (block-
                # transposed) so the gather back is a 3-dim AP.
                BB = CAP * CAP
                d_eo2 = dpool.tile([24, BB], f32, tag="d_eo2", name="d_eo2")
                d_ed = dpool.tile([128, CAP], f32, tag="d_ed", name="d_ed")
                for h in range(2):
                    ap_out = bass.AP(tensor=d_eo2.tensor, offset=h * 12 * BB,
                                     ap=[[CAP, CAP], [BB, 12], [1, CAP]])
                    dma2(out=ap_out, in_=E_own[h * 64:(h + 1) * 64, :])
                dma2(out=d_ed[:], in_=E_diag[:])

                t1src = big.tile([128, BB], f32, tag="t1src", name="t1src")
                ap_in = bass.AP(tensor=d_ed.tensor, offset=0,
                                ap=[[BB, 2], [0, 64], [1, BB]])
                dma2(out=t1src[:], in_=ap_in)
                # t3src halves at 0/64, k-major: row = h*64 + k*12 + t.
                # Dead rows (60-63, 124-127) get finite filler via tiny DMAs
                # (zero-scale rows must not hold NaN garbage).
                t3src = big.tile([128, BB], f32, tag="t3src", name="t3src")
                for h in range(2):
                    ap_in = bass.AP(tensor=d_eo2.tensor, offset=h * 12 * BB,
                                    ap=[[0, 5], [BB, 12], [1, BB]])
                    dma2(out=t3src[h * 64:h * 64 + 60, :], in_=ap_in)
                    ap_fill = bass.AP(tensor=d_eo2.tensor, offset=0,
                                      ap=[[0, 4], [1, BB]])
                    dma2(out=t3src[h * 64 + 60:h * 64 + 64, :], in_=ap_fill)

                # ---------------- k2 / k3 static builds ----------------
                ptrow2b = [big.tile([128, N], f32, tag=f"ptrow2b{q}",
                                    name=f"ptrow2b{q}") for q in range(2)]
                nc.gpsimd.partition_broadcast(ptrow2b[0][:], ptr2[0][:])
                nc.gpsimd.partition_broadcast(ptrow2b[1][:], ptr2[1][:])
                ptw3 = big.tile([128, N], f32, tag="ptw3", name="ptw3")
                ptw3t = big.tile([128, N], f32, tag="ptw3t", name="ptw3t")
                nc.gpsimd.partition_broadcast(ptw3[:], ptr3[0][:])
                nc.gpsimd.partition_broadcast(ptw3t[:], ptr3[1][:])
                nc.vector.tensor_copy(ptw3[CAP:128, :], ptw3t[CAP:128, :])

                k2P = []
                k2D = []
                for q in range(2):
                    P = big.tile([128, 3 * N], f32, tag=f"k2P{q}", name=f"k2P{q}")
                    colap = bass.AP(tensor=ptcolf.tensor,
                                    offset=ptcolf.offset + q * 3,
                                    ap=[list(ptcolf.ap[0]), [1, 3], [0, N]])
                    rowap = bass.AP(tensor=ptrow2b[q].tensor,
                                    offset=ptrow2b[q].offset,
                                    ap=[list(ptrow2b[q].ap[0]), [0, 3], [1, N]])
                    nc.vector.tensor_tensor(P[:], colap, rowap, OP.mult)
                    Dt = big.tile([128, 3 * N], f32, tag=f"k2D{q}", name=f"k2D{q}")
                    nc.vector.tensor_tensor(Dt[:], E_ttf[:], P[:], OP.mult)
                    k2P.append(P)
                    k2D.append(Dt)

                k3D = big.tile([128, N], f32, tag="k3D", name="k3D")
                nc.vector.tensor_tensor(k3D[:], E_pt[:], ptw3[:], OP.mult)

            if STAGE >= 40:
                # ---------------- gammas ----------------
                S1 = sred[:, 0:12]
                sttM = sred[:, 12:24]
                sstM = sred[:, 24:36]

                def diag_col(mat, nm):
                    s_ = scr.tile([C, C], f32, tag="diagscr", name="dsc")
                    col = sm.tile([C, 1], f32, tag=nm, name=nm)
                    nc.vector.tensor_tensor(s_[:], mat, eye12[:], OP.mult)
                    nc.vector.reduce_sum(out=col[:], in_=s_[:],
                                         axis=mybir.AxisListType.X)
                    return col

                ssscol = diag_col(ssall[:], "ssscol")
                sttcol = diag_col(sttM, "sttcol")
                sstd0 = diag_col(sstM, "sstd0")
                # combine own-class sst diag (from ssps) with the k2-class
                # entries recomputed locally (scksb)
                sstdcol = sm.tile([C, 1], f32, tag="sstdcol", name="sstdcol")
                nc.vector.tensor_tensor(sstdcol[:], sstd0[:], mab, OP.mult)
                nc.vector.tensor_tensor(sstdcol[:], sstdcol[:], scksb[:],
                                        OP.add)

                gin = sm.tile([C, 1], f32, tag="gin", name="gin")
                nc.vector.scalar_tensor_tensor(out=gin[:], in0=sstdcol[:], scalar=2.0,
                                               in1=sttcol[:], op0=OP.mult, op1=OP.add)
                nc.vector.tensor_tensor(gin[:], gin[:], ssscol[:], OP.add)
                nc.vector.tensor_tensor(gin[:], gin[:], rdenin[:], OP.mult)

                ssst = pT.tile([1, C], f32, tag="tiny", name="ssst")
                nc.tensor.transpose(ssst[:], ssscol[:], eye12[:])
                ssstsb = sm.tile([1, C], f32, tag="ssstsb", name="ssstsb")
                nc.vector.tensor_copy(ssstsb[:], ssst[:])
                sssrowb = sm.tile([C, C], f32, tag="sssrowb", name="sssrowb")
                nc.gpsimd.partition_broadcast(sssrowb[:], ssstsb[:])
                g2 = sm.tile([C, C], f32, tag="g2", name="g2")
                nc.vector.tensor_scalar(g2[:], S1, 2.0, None, OP.mult)
                nc.vector.tensor_tensor(g2[:], g2[:], sssrowb[:], OP.add)
                nc.vector.tensor_scalar(g2[:], g2[:], ssscol[:], None, OP.add)
                nc.vector.tensor_tensor(g2[:], g2[:], rden2[:], OP.mult)

                # IBG [12, 65] = -1/bw : cols 0-59 from g2 (k-major), 60-64 from gin
                ibg0 = sm.tile([C, 65], f32, tag="ibg0", name="ibg0")
                g2ap = g2[:]
                g2exp = bass.AP(tensor=g2ap.tensor, offset=g2ap.offset,
                                ap=[list(g2ap.ap[0]), [0, 5], [1, 12]])
                nc.vector.tensor_tensor(ibg0[:, 0:60], g2exp, pw60[:], OP.mult)
                ginap = gin[:]
                ginexp = bass.AP(tensor=ginap.tensor, offset=ginap.offset,
                                 ap=[list(ginap.ap[0]), [0, 5]])
                nc.vector.tensor_tensor(ibg0[:, 60:65], ginexp, pw5[:], OP.mult)
                nc.vector.tensor_scalar(ibg0[:], ibg0[:], -1e-5, None, OP.min)
                ibg = sm.tile([C, 65], f32, tag="ibg", name="ibg")
                nc.vector.reciprocal(ibg[:], ibg0[:])

                selsb = []
                for h in range(2):
                    ps_ = pT.tile([1, 65], f32, tag="tiny", name="psel")
                    nc.tensor.matmul(ps_[:], oh2[:, h:h + 1], ibg[:],
                                     start=True, stop=True)
                    s_ = sm.tile([1, 65], f32, tag=f"sel{h}", name=f"sel{h}")
                    nc.vector.tensor_copy(s_[:], ps_[:])
                    selsb.append(s_)

                sclT1 = sm.tile([128, 1], f32, tag="sclT1", name="sclT1")
                sclT3 = sm.tile([128, 1], f32, tag="sclT3", name="sclT3")
                nc.vector.memset(sclT1[:], 0.0)
                nc.vector.memset(sclT3[:], 0.0)
                negk1 = sm.tile([128, 5], f32, tag="negk1", name="negk1")
                for h in range(2):
                    tp_ = pT.tile([65, 1], f32, tag="tiny", name="tsel")
                    nc.tensor.transpose(tp_[:], selsb[h][:], eye128[0:1, 0:1])
                    tpsb = scr.tile([65, 1], f32, tag="tselsb", name="tpsb")
                    nc.vector.tensor_copy(tpsb[:], tp_[:])
                    nc.vector.tensor_copy(sclT1[h * 64:h * 64 + 60, :], tpsb[0:60, :])
                    pp_ = pT.tile([1, 65], f32, tag="tiny", name="pp_")
                    nc.tensor.matmul(pp_[:], tpsb[:], perm65[:], start=True, stop=True)
                    ppsb = scr.tile([1, 65], f32, tag="ppermsb", name="ppsb")
                    nc.vector.tensor_copy(ppsb[:], pp_[:])
                    tp2 = pT.tile([65, 1], f32, tag="tiny", name="tp2")
                    nc.tensor.transpose(tp2[:], ppsb[:], eye128[0:1, 0:1])
                    tp2sb = scr.tile([65, 1], f32, tag="tsel2sb", name="tp2sb")
                    nc.vector.tensor_copy(tp2sb[:], tp2[:])
                    nc.vector.tensor_copy(sclT3[h * 64:h * 64 + 60, :], tp2sb[0:60, :])
                    nkt = sm.tile([128, 5], f32, tag=f"negk1t{h}",
                                  name=f"nkt{h}")
                    nc.gpsimd.partition_broadcast(nkt[:], selsb[h][0:1, 60:65])
                    if h == 0:
                        nc.vector.tensor_copy(negk1[0:CAP, :], nkt[0:CAP, :])
                    else:
                        nc.vector.tensor_copy(negk1[CAP:128, :], nkt[CAP:128, :])

                negb = []
                for q in range(2):
                    k2sc = pT.tile([1, 5], f32, tag="tiny", name="k2sc")
                    nc.tensor.matmul(k2sc[:], k2sel[:, q:q + 1], ibg[:, 60:65],
                                     start=True, stop=True)
                    k2scsb = sm.tile([1, 5], f32, tag=f"k2scsb{q}", name=f"k2scsb{q}")
                    nc.vector.tensor_copy(k2scsb[:], k2sc[:])
                    nb = sm.tile([128, 5], f32, tag=f"negb{q}", name=f"negb{q}")
                    nc.gpsimd.partition_broadcast(nb[:], k2scsb[:])
                    negb.append(nb)

            if STAGE >= 50:
                # ---------------- ACC + exp passes ----------------
                acc = big.tile([128, NCOL], f32, tag="acc", name="acc")
                nc.vector.memset(acc[:], 0.0)

                nc.scalar.activation(t1src[:], t1src[:], AF.Exp, scale=sclT1[:],
                                     accum_out=acc[:, 0:1])
                nc.scalar.activation(t3src[:], t3src[:], AF.Exp,
                                     scale=sclT3[:], accum_out=acc[:, 1:2])

                for k in range(KN):
                    sk = scr.tile([128, CAP], f32, tag="k1scr", name="sk1")
                    nc.scalar.activation(sk[:], E_diag[:], AF.Exp,
                                         scale=negk1[:, k:k + 1],
                                         accum_out=acc[:, 2 + k:3 + k])

                for k in range(KN):
                    ek = scr.tile([128, N], f32, tag="k3e", name="ek3")
                    nc.scalar.activation(ek[:], k3D[:], AF.Exp,
                                         scale=negk1[:, k:k + 1])
                    sk = scr.tile([128, N], f32, tag="k3scr", name="sk3")
                    nc.vector.scalar_tensor_tensor(
                        out=sk[:], in0=ek[:], scalar=1.0, in1=ptw3[:],
                        op0=OP.mult, op1=OP.mult,
                        accum_out=acc[:, 7 + k:8 + k])

                for q in range(2):
                    for k in range(KN):
                        ek = scr.tile([128, 3 * N], f32, tag="k2e", name="ek2")
                        nc.scalar.activation(ek[:], k2D[q][:], AF.Exp,
                                             scale=negb[q][:, k:k + 1])
                        sk = scr.tile([128, 3 * N], f32, tag="k2scr",
                                      name="sk2")
                        col = 12 + q * KN + k
                        nc.vector.scalar_tensor_tensor(
                            out=sk[:], in0=ek[:], scalar=1.0, in1=k2P[q][:],
                            op0=OP.mult, op1=OP.mult,
                            accum_out=acc[:, col:col + 1])

                # ---------------- final weighted reduce ----------------
                v = big.tile([128, NCOL], f32, tag="v", name="v")
                nc.vector.tensor_tensor(v[:], acc[:], wm[:], OP.mult)
                m1 = pT.tile([NCOL, 1], f32, tag="tiny", name="m1")
                nc.tensor.matmul(m1[:], v[:], ones[:], start=True, stop=True)
                m1sb = sm.tile([NCOL, 1], f32, tag="m1sb", name="m1sb")
                nc.vector.tensor_copy(m1sb[:], m1[:])
                m2 = pT.tile([1, 2], f32, tag="tiny", name="m2")
                nc.tensor.matmul(m2[:], m1sb[:], ssel[:], start=True, stop=True)
                res = sm.tile([1, 2], f32, tag="res", name="res")
                nc.vector.tensor_tensor(res[:], m2[:], offs[:], OP.add)
                dma(out=o_out[:], in_=res[:])
            if STAGE < 50:
                dma(out=o_out[:], in_=wm[0:1, 0:2])

    nc.compile()
    return nc


def get_program():
    import os
    key = ("nc", os.environ.get("CDD_STAGE", "99"))
    if key not in _COMPILED:
        _COMPILED[key] = _build_program()
    return _COMPILED[key]


# ----------------------------------------------------------------------------
# entry point
# ----------------------------------------------------------------------------

def _run(in_maps, trace=False):
    from concourse.bass_utils import run_bass_kernel_spmd
    nc = get_program()
    return run_bass_kernel_spmd(nc, in_maps, list(range(NCORES)), trace=trace)


def kernel(src_x, tgt_x, src_y, tgt_y):
    in_maps = _host_prep(src_x, tgt_x, src_y, tgt_y)
    if in_maps is None:
        return _numpy_fallback(src_x, tgt_x, src_y, tgt_y)
    br = _run(in_maps)
    total = np.zeros(2, np.float64)
    for res in br.results:
        total += res["out"].reshape(2).astype(np.float64)
    return total.astype(np.float32)

